# revision 8
# baseline (speedup 1.0000x reference)
"""Multi-head causal attention (B=4, S=4096, E=512, H=8) on 8 trn2 NeuronCores.

Sharding: core = (batch b, head-group g of 4 heads); 4 batches x 2 groups = 8 cores.
Each core computes qkv projection for its group's heads, causal attention, and a
partial output projection (its heads' rows of Wo). Host sums the two partials per
batch and adds bo.

v2: bf16 operands everywhere (FWL weight loads, 2x SBUF/DMA traffic), QE=512
query sweeps with a merged e/o score tile [128, 1024] so each key-block
iteration is ONE exp activation; double-buffered score PSUM so QK(kb+1)
overlaps exp(kb); projection and Wo work interleaved into the attention
stream to fill PE gaps while the scalar engine (exp) saturates.

Device layout (per core):
  xT   [512, 4096] bf16   x[b] transposed -> contraction dim on partitions
  qT/kT stored [128(2 heads' dh), 1024-token tiles]
  V    stored token-major [128, kb*260 + h*65 + d] bf16 with a ones column per
       (kb, head) at d=64 -> PV matmul lhsT [Vh|1] yields attention output
       in [dh, tok] layout AND softmax denominators in one pass.
  st   [128 keys, 1024] PSUM per key-block: cols 0:512 = even head of the
       pair, 512:1024 = odd head (QK pair runs row-tile concurrent on PE);
       causal mask accumulated on PE via ident@maskT; ONE exp (scale=1/8
       folded) -> pt bf16; PV accumulates over key blocks in PSUM [65, 512].
"""

import sys

sys.path.insert(0, "/opt/trn_rl_repo")

import numpy as np

B, S, E = 4, 4096, 512
H = 8
DH = 64
HPG = 4  # heads per group
GQ = 256  # features per group for each of q/k/v (HPG*DH)
QE = 512  # query extent per attention sweep
NQ = S // QE  # 8
NTQ = 4  # token chunks for projection phase
TQ = S // NTQ  # 1024
VW = HPG * 65  # 260: per-key-block V width incl. ones columns
NEG = -1.0e10
SCALE = 0.125  # 1/sqrt(DH)

_CACHE = {}


def _build_nc():
    import concourse.bass as bass
    import concourse.tile as tile
    import concourse.mybir as mybir
    from concourse import bacc

    f32 = mybir.dt.float32
    bf = mybir.dt.bfloat16
    AF = mybir.ActivationFunctionType
    ALU = mybir.AluOpType

    nc = bacc.Bacc("TRN2", target_bir_lowering=False, debug=False)

    xT = nc.dram_tensor("xT", [E, S], bf, kind="ExternalInput").ap()
    wqk = nc.dram_tensor("wqk", [E, 512], bf, kind="ExternalInput").ap()
    bqk = nc.dram_tensor("bqk", [128, 4], f32, kind="ExternalInput").ap()
    wv = nc.dram_tensor("wv", [E, GQ], bf, kind="ExternalInput").ap()
    bv = nc.dram_tensor("bv", [1, GQ], bf, kind="ExternalInput").ap()
    wo = nc.dram_tensor("wo", [DH, HPG * 512], bf, kind="ExternalInput").ap()
    out = nc.dram_tensor("out", [S, E], f32, kind="ExternalOutput").ap()

    with tile.TileContext(nc) as tc:
        with (
            tc.tile_pool(name="consts", bufs=1) as cpool,
            tc.tile_pool(name="xt", bufs=2) as xtpool,
            tc.tile_pool(name="qkv", bufs=1) as qkvpool,
            tc.tile_pool(name="pt", bufs=3) as ptpool,
            tc.tile_pool(name="att", bufs=2) as attpool,
            tc.tile_pool(name="eps", bufs=2) as epool,
            tc.tile_pool(name="outs", bufs=2) as opool,
            # PSUM budget (8 banks of 512 f32):
            #   st  [128,1024] x2 bufs = 4 banks
            #   ov_e/ov_o [65,512] x1  = 2 banks
            #   aux [128,512] x2 bufs  = 2 banks (proj + wo matmul groups)
            tc.tile_pool(name="st", bufs=2, space="PSUM") as stpool,
            tc.tile_pool(name="ov", bufs=1, space="PSUM") as ovpool,
            tc.tile_pool(name="aux", bufs=2, space="PSUM") as auxpool,
        ):
            # ---- constants ----
            wqk_sb = cpool.tile([128, 4 * 512], bf, name="wqk_sb")
            for ec in range(4):
                nc.sync.dma_start(
                    wqk_sb[:, ec * 512 : (ec + 1) * 512],
                    wqk[ec * 128 : (ec + 1) * 128, :],
                )
            wv_sb = cpool.tile([128, 4 * GQ], bf, name="wv_sb")
            for ec in range(4):
                nc.sync.dma_start(
                    wv_sb[:, ec * GQ : (ec + 1) * GQ],
                    wv[ec * 128 : (ec + 1) * 128, :],
                )
            wo_sb = cpool.tile([DH, HPG * 512], bf, name="wo_sb")
            nc.sync.dma_start(wo_sb[:], wo[:])
            bqk_sb = cpool.tile([128, 4], f32, name="bqk_sb")
            nc.sync.dma_start(bqk_sb[:], bqk[:])
            bv_sb = cpool.tile([1, GQ], bf, name="bv_sb")
            nc.sync.dma_start(bv_sb[:], bv[:])
            onesf = cpool.tile([128, 128], bf, name="onesf")
            nc.vector.memset(onesf[:], 1.0)
            ones_row = cpool.tile([1, 128], bf, name="ones_row")
            nc.vector.tensor_copy(ones_row[:], onesf[0:1, :])
            maskf = cpool.tile([128, 128], f32, name="maskf")
            nc.vector.memset(maskf[:], 0.0)
            nc.gpsimd.affine_select(
                out=maskf[:], in_=maskf[:], compare_op=ALU.is_ge, fill=NEG,
                base=0, pattern=[[1, 128]], channel_multiplier=-1,
            )
            maskT = cpool.tile([128, 128], bf, name="maskT")
            nc.vector.tensor_copy(maskT[:], maskf[:])
            identf = cpool.tile([128, 128], f32, name="identf")
            nc.vector.memset(identf[:], 0.0)
            nc.gpsimd.affine_select(
                out=identf[:], in_=identf[:], compare_op=ALU.not_equal, fill=1.0,
                base=0, pattern=[[-1, 128]], channel_multiplier=1,
            )
            ident = cpool.tile([128, 128], bf, name="ident")
            nc.vector.tensor_copy(ident[:], identf[:])

            # persistent qT/kT tiles: [pair A/B][tq] each [128, 1024] bf16
            # pair A rows 0:64 = head0 dh, 64:128 = head1; pair B = heads 2,3
            qt = [
                [qkvpool.tile([128, TQ], bf, name=f"qt{ab}_{t}") for t in range(NTQ)]
                for ab in range(2)
            ]
            kt = [
                [qkvpool.tile([128, TQ], bf, name=f"kt{ab}_{t}") for t in range(NTQ)]
                for ab in range(2)
            ]
            vt = [qkvpool.tile([128, 8 * VW], bf, name=f"vt_{t}") for t in range(NTQ)]

            # deferred-work queue: small PE work quanta (projection psum groups,
            # Wo psum groups) emitted one per attention kb-iteration so the PE
            # does them while the scalar engine churns exps, instead of in a
            # serial block at sweep boundaries (which idles ACT ~14us each).
            work_q = []

            def drain_one():
                if work_q:
                    work_q.pop(0)()

            def drain_all():
                while work_q:
                    work_q.pop(0)()

            def p1(tq, defer):
                xts = []
                for ec in range(4):
                    xtile = xtpool.tile([128, TQ], bf, name="xtile", tag=f"xt{ec}")
                    nc.sync.dma_start(
                        xtile[:],
                        xT[ec * 128 : (ec + 1) * 128, tq * TQ : (tq + 1) * TQ],
                    )
                    xts.append(xtile)

                def qk_chunk(fc, th):
                    dest = (qt if fc < 2 else kt)[fc % 2][tq]
                    ps = auxpool.tile([128, 512], f32, name="pjps", tag="aux")
                    for ec in range(4):
                        nc.tensor.matmul(
                            ps[:],
                            lhsT=wqk_sb[:, ec * 512 + fc * 128 : ec * 512 + (fc + 1) * 128],
                            rhs=xts[ec][:, th * 512 : (th + 1) * 512],
                            start=(ec == 0),
                            stop=(ec == 3),
                        )
                    nc.vector.tensor_scalar_add(
                        dest[:, th * 512 : (th + 1) * 512],
                        ps[:],
                        bqk_sb[:, fc : fc + 1],
                    )

                def v_ones():
                    nc.vector.tensor_copy(
                        vt[tq].rearrange("p (t h d) -> p t h d", t=8, h=HPG)[:, :, :, 64:65],
                        onesf[:, 0:32].rearrange("p (t h d) -> p t h d", t=8, h=HPG),
                    )

                def v_chunk(tb):
                    vps = auxpool.tile([128, GQ], f32, name="vps", tag="aux")
                    for ec in range(4):
                        nc.tensor.matmul(
                            vps[:],
                            lhsT=xts[ec][:, tb * 128 : (tb + 1) * 128],
                            rhs=wv_sb[:, ec * GQ : (ec + 1) * GQ],
                            start=(ec == 0),
                            stop=False,
                        )
                    nc.tensor.matmul(
                        vps[:], lhsT=ones_row[:], rhs=bv_sb[:], start=False, stop=True
                    )
                    nc.vector.tensor_copy(
                        vt[tq][:, tb * VW : (tb + 1) * VW].rearrange(
                            "p (h d) -> p h d", h=HPG
                        )[:, :, 0:64],
                        vps.rearrange("p (h d) -> p h d", h=HPG),
                    )

                chunks = [v_ones]
                chunks += [
                    (lambda fc=fc, th=th: qk_chunk(fc, th))
                    for fc in range(4)
                    for th in range(2)
                ]
                chunks += [(lambda tb=tb: v_chunk(tb)) for tb in range(8)]
                if defer:
                    work_q.extend(chunks)
                else:
                    for c in chunks:
                        c()

            atts = {}

            def att(qq, pr):
                nkb = 4 * qq + 4
                tqq, qoff = qq // 2, (qq % 2) * QE
                ov_e = ovpool.tile([65, QE], f32, name="ov_e", tag="ov_e")
                ov_o = ovpool.tile([65, QE], f32, name="ov_o", tag="ov_o")

                def pv(kb, pt, qs):
                    tqk, kbl = kb // 8, kb % 8
                    nc.tensor.matmul(
                        ov_e[:, qs:QE],
                        lhsT=vt[tqk][:, kbl * VW + 2 * pr * 65 : kbl * VW + (2 * pr + 1) * 65],
                        rhs=pt[:, qs:QE],
                        start=(kb == 0),
                        stop=(kb == nkb - 1),
                        skip_group_check=True,
                    )
                    nc.tensor.matmul(
                        ov_o[:, qs:QE],
                        lhsT=vt[tqk][:, kbl * VW + (2 * pr + 1) * 65 : kbl * VW + (2 * pr + 2) * 65],
                        rhs=pt[:, QE + qs : 2 * QE],
                        start=(kb == 0),
                        stop=(kb == nkb - 1),
                        skip_group_check=True,
                    )

                prev = None
                for kb in range(nkb):
                    tqk, kbl = kb // 8, kb % 8
                    qs = max(0, kb * 128 - qq * QE)
                    diag = kb >= 4 * qq
                    st = stpool.tile([128, 2 * QE], f32, name="st", tag="st")
                    # QK^T for the head pair: rows 0:64 (even) and 64:128 (odd)
                    # use disjoint PE row groups -> concurrent matmuls.
                    nc.tensor.matmul(
                        st[:, qs:QE],
                        lhsT=kt[pr][tqk][0:64, kbl * 128 : (kbl + 1) * 128],
                        rhs=qt[pr][tqq][0:64, qoff + qs : qoff + QE],
                        start=True,
                        stop=not diag,
                    )
                    nc.tensor.matmul(
                        st[:, QE + qs : 2 * QE],
                        lhsT=kt[pr][tqk][64:128, kbl * 128 : (kbl + 1) * 128],
                        rhs=qt[pr][tqq][64:128, qoff + qs : qoff + QE],
                        start=True,
                        stop=not diag,
                    )
                    if diag:
                        for half in range(2):
                            nc.tensor.matmul(
                                st[:, half * QE + qs : half * QE + qs + 128],
                                lhsT=ident[:],
                                rhs=maskT[:],
                                start=False,
                                stop=True,
                                skip_group_check=True,
                            )
                    pt = ptpool.tile([128, 2 * QE], bf, name="pt", tag="pt")
                    if qs == 0:
                        nc.scalar.activation(
                            pt[:], st[:], AF.Exp, bias=0.0, scale=SCALE
                        )
                    else:
                        st3 = st.rearrange("p (t c) -> p t c", t=2)[:, :, qs:QE]
                        pt3 = pt.rearrange("p (t c) -> p t c", t=2)[:, :, qs:QE]
                        nc.scalar.activation(pt3, st3, AF.Exp, bias=0.0, scale=SCALE)
                    # software pipeline: emit PV for the PREVIOUS kb after this
                    # kb's QK+exp, so the in-order PE queue reaches QK(kb+1)
                    # without stalling on exp(kb).
                    if prev is not None:
                        pv(*prev)
                    prev = (kb, pt, qs)
                    drain_one()
                pv(*prev)
                # epilogue: copy denominators to SBUF (both heads into one
                # [1, 2*QE] tile), one batched reciprocal, then fused
                # normalize+cast per head.
                den_sb = epool.tile([1, 2 * QE], f32, name="den", tag=f"den{pr}")
                nc.vector.tensor_copy(den_sb[:, 0:QE], ov_e[64:65, :])
                nc.vector.tensor_copy(den_sb[:, QE : 2 * QE], ov_o[64:65, :])
                rec = epool.tile([1, 2 * QE], f32, name="rec", tag=f"rec{pr}")
                scr = epool.tile([1, 2 * QE], f32, name="scr", tag=f"scr{pr}")
                nc.vector.reciprocal_approx_accurate(
                    out=rec[:], in_=den_sb[:], scratch=scr[:]
                )
                for half, ov in ((0, ov_e), (1, ov_o)):
                    h = 2 * pr + half
                    ah = attpool.tile([DH, QE], bf, name=f"att{h}", tag=f"att{h}")
                    rb = epool.tile([DH, QE], f32, name="rb", tag=f"rb{h}")
                    nc.sync.dma_start(
                        rb[:],
                        rec[:, half * QE : (half + 1) * QE]
                        .unsqueeze(1)
                        .to_broadcast([1, DH, QE]),
                    )
                    nc.vector.tensor_tensor(ah[:], ov[0:DH, :], rb[:], ALU.mult)
                    atts[(qq, h)] = ah

            def wo_out(qq, defer):
                out_sb = opool.tile([128, 4 * 512], f32, name="out_sb", tag="osb")

                def wo_chunk(tb4):
                    wops = auxpool.tile([128, 512], f32, name="wops", tag="aux")
                    for h in range(HPG):
                        nc.tensor.matmul(
                            wops[:],
                            lhsT=atts[(qq, h)][:, tb4 * 128 : (tb4 + 1) * 128],
                            rhs=wo_sb[:, h * 512 : (h + 1) * 512],
                            start=(h == 0),
                            stop=(h == HPG - 1),
                        )
                    nc.vector.tensor_copy(out_sb[:, tb4 * 512 : (tb4 + 1) * 512], wops[:])

                def wo_dma():
                    nc.sync.dma_start(
                        out[qq * QE : (qq + 1) * QE, :].rearrange("(t p) c -> p t c", p=128),
                        out_sb.rearrange("p (t c) -> p t c", t=4),
                    )

                chunks = [(lambda tb4=tb4: wo_chunk(tb4)) for tb4 in range(4)] + [wo_dma]
                if defer:
                    work_q.extend(chunks)
                else:
                    for c in chunks:
                        c()

            p1(0, defer=False)
            for qq in range(NQ):
                if qq < NTQ - 1:
                    p1(qq + 1, defer=True)
                att(qq, 0)
                att(qq, 1)
                wo_out(qq, defer=(qq < NQ - 1))
            drain_all()

    nc.finalize()
    return nc


def _get_nc():
    if "nc" not in _CACHE:
        _CACHE["nc"] = _build_nc()
    return _CACHE["nc"]


def _make_in_maps(x, Wqkv, bqkv, Wo):
    import ml_dtypes

    bf16 = ml_dtypes.bfloat16
    in_maps = []
    for core in range(8):
        b, g = core // 2, core % 2
        qs, ks, vs = g * GQ, 512 + g * GQ, 1024 + g * GQ
        wqk_np = np.ascontiguousarray(
            np.concatenate([Wqkv[:, qs : qs + GQ], Wqkv[:, ks : ks + GQ]], axis=1)
        ).astype(bf16)
        bqk_np = np.ascontiguousarray(
            np.concatenate([bqkv[qs : qs + GQ], bqkv[ks : ks + GQ]]).reshape(4, 128).T
        )
        wv_np = np.ascontiguousarray(Wqkv[:, vs : vs + GQ]).astype(bf16)
        bv_np = np.ascontiguousarray(bqkv[vs : vs + GQ].reshape(1, GQ)).astype(bf16)
        wo_g = Wo[g * GQ : (g + 1) * GQ, :]
        wo_np = np.ascontiguousarray(
            np.concatenate([wo_g[h * DH : (h + 1) * DH, :] for h in range(HPG)], axis=1)
        ).astype(bf16)
        in_maps.append(
            {
                "xT": np.ascontiguousarray(x[b].T).astype(bf16),
                "wqk": wqk_np,
                "bqk": bqk_np,
                "wv": wv_np,
                "bv": bv_np,
                "wo": wo_np,
            }
        )
    return in_maps


def kernel(x, Wqkv, bqkv, Wo, bo, **run_kwargs):
    from concourse.bass_utils import run_bass_kernel_spmd

    x = np.asarray(x, dtype=np.float32)
    Wqkv = np.asarray(Wqkv, dtype=np.float32)
    bqkv = np.asarray(bqkv, dtype=np.float32)
    Wo = np.asarray(Wo, dtype=np.float32)
    bo = np.asarray(bo, dtype=np.float32)

    nc = _get_nc()
    in_maps = _make_in_maps(x, Wqkv, bqkv, Wo)

    res = run_bass_kernel_spmd(nc, in_maps, core_ids=list(range(8)), **run_kwargs)
    _CACHE["last_results"] = res

    out = np.empty((B, S, E), dtype=np.float32)
    for b in range(B):
        out[b] = res.results[2 * b]["out"] + res.results[2 * b + 1]["out"] + bo
    return out


# revision 13
# speedup vs baseline: 1.0103x; 1.0103x over previous
"""Multi-head causal attention (B=4, S=4096, E=512, H=8) on 8 trn2 NeuronCores.

Sharding: core = (batch b, head-group g of 4 heads); 4 batches x 2 groups = 8 cores.
Each core computes qkv projection for its group's heads, causal attention, and a
partial output projection (its heads' rows of Wo). Host sums the two partials per
batch and adds bo.

v2: bf16 operands everywhere (FWL weight loads, 2x SBUF/DMA traffic), QE=512
query sweeps with a merged e/o score tile [128, 1024] so each key-block
iteration is ONE exp activation; double-buffered score PSUM so QK(kb+1)
overlaps exp(kb); projection and Wo work interleaved into the attention
stream to fill PE gaps while the scalar engine (exp) saturates.

Device layout (per core):
  xT   [512, 4096] bf16   x[b] transposed -> contraction dim on partitions
  qT/kT stored [128(2 heads' dh), 1024-token tiles]
  V    stored token-major [128, kb*260 + h*65 + d] bf16 with a ones column per
       (kb, head) at d=64 -> PV matmul lhsT [Vh|1] yields attention output
       in [dh, tok] layout AND softmax denominators in one pass.
  st   [128 keys, 1024] PSUM per key-block: cols 0:512 = even head of the
       pair, 512:1024 = odd head (QK pair runs row-tile concurrent on PE);
       causal mask accumulated on PE via ident@maskT; ONE exp (scale=1/8
       folded) -> pt bf16; PV accumulates over key blocks in PSUM [65, 512].
"""

import sys

sys.path.insert(0, "/opt/trn_rl_repo")

import numpy as np

B, S, E = 4, 4096, 512
H = 8
DH = 64
HPG = 4  # heads per group
GQ = 256  # features per group for each of q/k/v (HPG*DH)
QE = 512  # query extent per attention sweep
NQ = S // QE  # 8
NTQ = 4  # token chunks for projection phase
TQ = S // NTQ  # 1024
VW = HPG * 65  # 260: per-key-block V width incl. ones columns
NEG = -1.0e10
SCALE = 0.125  # 1/sqrt(DH)

_CACHE = {}


def _build_nc():
    import concourse.bass as bass
    import concourse.tile as tile
    import concourse.mybir as mybir
    from concourse import bacc

    f32 = mybir.dt.float32
    bf = mybir.dt.bfloat16
    AF = mybir.ActivationFunctionType
    ALU = mybir.AluOpType

    nc = bacc.Bacc("TRN2", target_bir_lowering=False, debug=False)

    xT = nc.dram_tensor("xT", [E, S], bf, kind="ExternalInput").ap()
    wqk = nc.dram_tensor("wqk", [E, 512], bf, kind="ExternalInput").ap()
    bqk = nc.dram_tensor("bqk", [128, 4], f32, kind="ExternalInput").ap()
    wv = nc.dram_tensor("wv", [E, GQ], bf, kind="ExternalInput").ap()
    bv = nc.dram_tensor("bv", [1, GQ], bf, kind="ExternalInput").ap()
    wo = nc.dram_tensor("wo", [DH, HPG * 512], bf, kind="ExternalInput").ap()
    out = nc.dram_tensor("out", [S, E], f32, kind="ExternalOutput").ap()

    with tile.TileContext(nc) as tc:
        with (
            tc.tile_pool(name="consts", bufs=1) as cpool,
            tc.tile_pool(name="xt", bufs=2) as xtpool,
            tc.tile_pool(name="qkv", bufs=1) as qkvpool,
            tc.tile_pool(name="pt", bufs=3) as ptpool,
            tc.tile_pool(name="att", bufs=2) as attpool,
            tc.tile_pool(name="eps", bufs=2) as epool,
            tc.tile_pool(name="outs", bufs=2) as opool,
            # PSUM budget (8 banks of 512 f32):
            #   st  [128,1024] x2 bufs = 4 banks
            #   ov_e/ov_o [65,512] x1  = 2 banks
            #   aux [128,512] x2 bufs  = 2 banks (proj + wo matmul groups)
            tc.tile_pool(name="st", bufs=2, space="PSUM") as stpool,
            tc.tile_pool(name="ov", bufs=1, space="PSUM") as ovpool,
            tc.tile_pool(name="aux", bufs=2, space="PSUM") as auxpool,
        ):
            # ---- constants ----
            wqk_sb = cpool.tile([128, 4 * 512], bf, name="wqk_sb")
            for ec in range(4):
                nc.sync.dma_start(
                    wqk_sb[:, ec * 512 : (ec + 1) * 512],
                    wqk[ec * 128 : (ec + 1) * 128, :],
                )
            wv_sb = cpool.tile([128, 4 * GQ], bf, name="wv_sb")
            for ec in range(4):
                nc.sync.dma_start(
                    wv_sb[:, ec * GQ : (ec + 1) * GQ],
                    wv[ec * 128 : (ec + 1) * 128, :],
                )
            wo_sb = cpool.tile([DH, HPG * 512], bf, name="wo_sb")
            nc.sync.dma_start(wo_sb[:], wo[:])
            bqk_sb = cpool.tile([128, 4], f32, name="bqk_sb")
            nc.sync.dma_start(bqk_sb[:], bqk[:])
            bv_sb = cpool.tile([1, GQ], bf, name="bv_sb")
            nc.sync.dma_start(bv_sb[:], bv[:])
            onesf = cpool.tile([128, 128], bf, name="onesf")
            nc.vector.memset(onesf[:], 1.0)
            ones_row = cpool.tile([1, 128], bf, name="ones_row")
            nc.vector.tensor_copy(ones_row[:], onesf[0:1, :])
            maskf = cpool.tile([128, 128], f32, name="maskf")
            nc.vector.memset(maskf[:], 0.0)
            nc.gpsimd.affine_select(
                out=maskf[:], in_=maskf[:], compare_op=ALU.is_ge, fill=NEG,
                base=0, pattern=[[1, 128]], channel_multiplier=-1,
            )
            maskT = cpool.tile([128, 128], bf, name="maskT")
            nc.vector.tensor_copy(maskT[:], maskf[:])
            identf = cpool.tile([128, 128], f32, name="identf")
            nc.vector.memset(identf[:], 0.0)
            nc.gpsimd.affine_select(
                out=identf[:], in_=identf[:], compare_op=ALU.not_equal, fill=1.0,
                base=0, pattern=[[-1, 128]], channel_multiplier=1,
            )
            ident = cpool.tile([128, 128], bf, name="ident")
            nc.vector.tensor_copy(ident[:], identf[:])

            # persistent qT/kT tiles: [pair A/B][tq] each [128, 1024] bf16
            # pair A rows 0:64 = head0 dh, 64:128 = head1; pair B = heads 2,3
            qt = [
                [qkvpool.tile([128, TQ], bf, name=f"qt{ab}_{t}") for t in range(NTQ)]
                for ab in range(2)
            ]
            kt = [
                [qkvpool.tile([128, TQ], bf, name=f"kt{ab}_{t}") for t in range(NTQ)]
                for ab in range(2)
            ]
            vt = [qkvpool.tile([128, 8 * VW], bf, name=f"vt_{t}") for t in range(NTQ)]

            # deferred-work queue: small PE work quanta (projection psum groups,
            # Wo psum groups) emitted one per attention kb-iteration so the PE
            # does them while the scalar engine churns exps, instead of in a
            # serial block at sweep boundaries (which idles ACT ~14us each).
            work_q = []  # entries: (label, thunk)

            def drain_one():
                if work_q:
                    work_q.pop(0)[1]()

            def drain_all():
                while work_q:
                    work_q.pop(0)[1]()

            def force_drain(label):
                # emit every queued chunk up to and including the last one
                # with this label (FIFO order preserved for correctness).
                while any(lbl == label for lbl, _ in work_q):
                    work_q.pop(0)[1]()

            def p1(tq, defer):
                xts = []
                for ec in range(4):
                    xtile = xtpool.tile([128, TQ], bf, name="xtile", tag=f"xt{ec}")
                    nc.sync.dma_start(
                        xtile[:],
                        xT[ec * 128 : (ec + 1) * 128, tq * TQ : (tq + 1) * TQ],
                    )
                    xts.append(xtile)

                def qk_chunk(fc, th):
                    dest = (qt if fc < 2 else kt)[fc % 2][tq]
                    ps = auxpool.tile([128, 512], f32, name="pjps", tag="aux")
                    for ec in range(4):
                        nc.tensor.matmul(
                            ps[:],
                            lhsT=wqk_sb[:, ec * 512 + fc * 128 : ec * 512 + (fc + 1) * 128],
                            rhs=xts[ec][:, th * 512 : (th + 1) * 512],
                            start=(ec == 0),
                            stop=(ec == 3),
                        )
                    nc.vector.tensor_scalar_add(
                        dest[:, th * 512 : (th + 1) * 512],
                        ps[:],
                        bqk_sb[:, fc : fc + 1],
                    )

                def v_ones():
                    nc.vector.tensor_copy(
                        vt[tq].rearrange("p (t h d) -> p t h d", t=8, h=HPG)[:, :, :, 64:65],
                        onesf[:, 0:32].rearrange("p (t h d) -> p t h d", t=8, h=HPG),
                    )

                def v_chunk(tb):
                    vps = auxpool.tile([128, GQ], f32, name="vps", tag="aux")
                    for ec in range(4):
                        nc.tensor.matmul(
                            vps[:],
                            lhsT=xts[ec][:, tb * 128 : (tb + 1) * 128],
                            rhs=wv_sb[:, ec * GQ : (ec + 1) * GQ],
                            start=(ec == 0),
                            stop=False,
                        )
                    nc.tensor.matmul(
                        vps[:], lhsT=ones_row[:], rhs=bv_sb[:], start=False, stop=True
                    )
                    nc.vector.tensor_copy(
                        vt[tq][:, tb * VW : (tb + 1) * VW].rearrange(
                            "p (h d) -> p h d", h=HPG
                        )[:, :, 0:64],
                        vps.rearrange("p (h d) -> p h d", h=HPG),
                    )

                chunks = [v_ones]
                chunks += [
                    (lambda fc=fc, th=th: qk_chunk(fc, th))
                    for fc in range(4)
                    for th in range(2)
                ]
                chunks += [(lambda tb=tb: v_chunk(tb)) for tb in range(8)]
                if defer:
                    work_q.extend((f"p1_{tq}", c) for c in chunks)
                else:
                    for c in chunks:
                        c()

            atts = {}

            def pv(kb, pt, qs, pr, nkb, ov_e, ov_o):
                tqk, kbl = kb // 8, kb % 8
                nc.tensor.matmul(
                    ov_e[:, qs:QE],
                    lhsT=vt[tqk][:, kbl * VW + 2 * pr * 65 : kbl * VW + (2 * pr + 1) * 65],
                    rhs=pt[:, qs:QE],
                    start=(kb == 0),
                    stop=(kb == nkb - 1),
                    skip_group_check=True,
                )
                nc.tensor.matmul(
                    ov_o[:, qs:QE],
                    lhsT=vt[tqk][:, kbl * VW + (2 * pr + 1) * 65 : kbl * VW + (2 * pr + 2) * 65],
                    rhs=pt[:, QE + qs : 2 * QE],
                    start=(kb == 0),
                    stop=(kb == nkb - 1),
                    skip_group_check=True,
                )

            def epilogue_chunks(qq, pr, ov_e, ov_o):
                # normalize by the denominators in row 64; split in two chunks
                # so they drain quickly at the head of the work queue.
                def c1():
                    den_sb = epool.tile([1, 2 * QE], f32, name="den", tag=f"den{pr}")
                    nc.vector.tensor_copy(den_sb[:, 0:QE], ov_e[64:65, :])
                    nc.vector.tensor_copy(den_sb[:, QE : 2 * QE], ov_o[64:65, :])
                    rec = epool.tile([1, 2 * QE], f32, name="rec", tag=f"rec{pr}")
                    scr = epool.tile([1, 2 * QE], f32, name="scr", tag=f"scr{pr}")
                    nc.vector.reciprocal_approx_accurate(
                        out=rec[:], in_=den_sb[:], scratch=scr[:]
                    )
                    atts[("rec", pr)] = rec

                def c2():
                    rec = atts[("rec", pr)]
                    for half, ov in ((0, ov_e), (1, ov_o)):
                        h = 2 * pr + half
                        ah = attpool.tile([DH, QE], bf, name=f"att{h}", tag=f"att{h}")
                        rb = epool.tile([DH, QE], f32, name="rb", tag=f"rb{h}")
                        nc.sync.dma_start(
                            rb[:],
                            rec[:, half * QE : (half + 1) * QE]
                            .unsqueeze(1)
                            .to_broadcast([1, DH, QE]),
                        )
                        nc.vector.tensor_tensor(ah[:], ov[0:DH, :], rb[:], ALU.mult)
                        atts[(qq, h)] = ah

                return [c1, c2]

            def wo_chunks(qq):
                out_sb = opool.tile([128, 4 * 512], f32, name="out_sb", tag="osb")

                def wo_chunk(tb4):
                    wops = auxpool.tile([128, 512], f32, name="wops", tag="aux")
                    for h in range(HPG):
                        nc.tensor.matmul(
                            wops[:],
                            lhsT=atts[(qq, h)][:, tb4 * 128 : (tb4 + 1) * 128],
                            rhs=wo_sb[:, h * 512 : (h + 1) * 512],
                            start=(h == 0),
                            stop=(h == HPG - 1),
                        )
                    nc.vector.tensor_copy(out_sb[:, tb4 * 512 : (tb4 + 1) * 512], wops[:])

                def wo_dma():
                    nc.sync.dma_start(
                        out[qq * QE : (qq + 1) * QE, :].rearrange("(t p) c -> p t c", p=128),
                        out_sb.rearrange("p (t c) -> p t c", t=4),
                    )

                return [(lambda tb4=tb4: wo_chunk(tb4)) for tb4 in range(4)] + [wo_dma]

            # One flat stream over all (qq, pr, kb) iterations. The PV matmul
            # is software-pipelined one iteration behind (crossing sweep
            # boundaries), so the in-order PE queue always reaches the next
            # QK -- and the scalar engine's exp stream never starves.
            p1(0, defer=False)
            prev = None
            for qq in range(NQ):
                for pr in range(2):
                    nkb = 4 * qq + 4
                    tqq, qoff = qq // 2, (qq % 2) * QE
                    ov_e = ovpool.tile([65, QE], f32, name="ov_e", tag="ov_e")
                    ov_o = ovpool.tile([65, QE], f32, name="ov_o", tag="ov_o")
                    if pr == 0:
                        # prefetch the NEXT projection chunk set; force-drain
                        # any leftovers of the one this sweep starts reading.
                        tq_next = qq // 2 + 1
                        if qq % 2 == 0 and tq_next < NTQ:
                            p1(tq_next, defer=True)
                        if qq % 2 == 0 and qq > 0:
                            force_drain(f"p1_{qq // 2}")
                    for kb in range(nkb):
                        tqk, kbl = kb // 8, kb % 8
                        qs = max(0, kb * 128 - qq * QE)
                        diag = kb >= 4 * qq
                        st = stpool.tile([128, 2 * QE], f32, name="st", tag="st")
                        # QK^T for the head pair: rows 0:64 (even) / 64:128
                        # (odd) use disjoint PE row groups -> concurrent.
                        nc.tensor.matmul(
                            st[:, qs:QE],
                            lhsT=kt[pr][tqk][0:64, kbl * 128 : (kbl + 1) * 128],
                            rhs=qt[pr][tqq][0:64, qoff + qs : qoff + QE],
                            start=True,
                            stop=not diag,
                        )
                        nc.tensor.matmul(
                            st[:, QE + qs : 2 * QE],
                            lhsT=kt[pr][tqk][64:128, kbl * 128 : (kbl + 1) * 128],
                            rhs=qt[pr][tqq][64:128, qoff + qs : qoff + QE],
                            start=True,
                            stop=not diag,
                        )
                        if diag:
                            for half in range(2):
                                nc.tensor.matmul(
                                    st[:, half * QE + qs : half * QE + qs + 128],
                                    lhsT=ident[:],
                                    rhs=maskT[:],
                                    start=False,
                                    stop=True,
                                    skip_group_check=True,
                                )
                        pt = ptpool.tile([128, 2 * QE], bf, name="pt", tag="pt")
                        if qs == 0:
                            nc.scalar.activation(
                                pt[:], st[:], AF.Exp, bias=0.0, scale=SCALE
                            )
                        else:
                            st3 = st.rearrange("p (t c) -> p t c", t=2)[:, :, qs:QE]
                            pt3 = pt.rearrange("p (t c) -> p t c", t=2)[:, :, qs:QE]
                            nc.scalar.activation(
                                pt3, st3, AF.Exp, bias=0.0, scale=SCALE
                            )
                        if prev is not None:
                            pv(*prev[0])
                            if prev[1] is not None:
                                # prev iteration closed a sweep: queue its
                                # epilogue (front) + Wo projection (back).
                                eqq, epr, eov_e, eov_o = prev[1]
                                for i, c in enumerate(
                                    epilogue_chunks(eqq, epr, eov_e, eov_o)
                                ):
                                    work_q.insert(i, ("ep", c))
                                if epr == 1:
                                    work_q.extend(
                                        ("wo", c) for c in wo_chunks(eqq)
                                    )
                        done = (
                            (qq, pr, ov_e, ov_o) if kb == nkb - 1 else None
                        )
                        prev = ((kb, pt, qs, pr, nkb, ov_e, ov_o), done)
                        drain_one()
            pv(*prev[0])
            eqq, epr, eov_e, eov_o = prev[1]
            for c in epilogue_chunks(eqq, epr, eov_e, eov_o):
                c()
            for c in wo_chunks(eqq):
                c()
            drain_all()

    nc.finalize()
    return nc


def _get_nc():
    if "nc" not in _CACHE:
        _CACHE["nc"] = _build_nc()
    return _CACHE["nc"]


def _make_in_maps(x, Wqkv, bqkv, Wo):
    import ml_dtypes

    bf16 = ml_dtypes.bfloat16
    in_maps = []
    for core in range(8):
        b, g = core // 2, core % 2
        qs, ks, vs = g * GQ, 512 + g * GQ, 1024 + g * GQ
        wqk_np = np.ascontiguousarray(
            np.concatenate([Wqkv[:, qs : qs + GQ], Wqkv[:, ks : ks + GQ]], axis=1)
        ).astype(bf16)
        bqk_np = np.ascontiguousarray(
            np.concatenate([bqkv[qs : qs + GQ], bqkv[ks : ks + GQ]]).reshape(4, 128).T
        )
        wv_np = np.ascontiguousarray(Wqkv[:, vs : vs + GQ]).astype(bf16)
        bv_np = np.ascontiguousarray(bqkv[vs : vs + GQ].reshape(1, GQ)).astype(bf16)
        wo_g = Wo[g * GQ : (g + 1) * GQ, :]
        wo_np = np.ascontiguousarray(
            np.concatenate([wo_g[h * DH : (h + 1) * DH, :] for h in range(HPG)], axis=1)
        ).astype(bf16)
        in_maps.append(
            {
                "xT": np.ascontiguousarray(x[b].T).astype(bf16),
                "wqk": wqk_np,
                "bqk": bqk_np,
                "wv": wv_np,
                "bv": bv_np,
                "wo": wo_np,
            }
        )
    return in_maps


def kernel(x, Wqkv, bqkv, Wo, bo, **run_kwargs):
    from concourse.bass_utils import run_bass_kernel_spmd

    x = np.asarray(x, dtype=np.float32)
    Wqkv = np.asarray(Wqkv, dtype=np.float32)
    bqkv = np.asarray(bqkv, dtype=np.float32)
    Wo = np.asarray(Wo, dtype=np.float32)
    bo = np.asarray(bo, dtype=np.float32)

    nc = _get_nc()
    in_maps = _make_in_maps(x, Wqkv, bqkv, Wo)

    res = run_bass_kernel_spmd(nc, in_maps, core_ids=list(range(8)), **run_kwargs)
    _CACHE["last_results"] = res

    out = np.empty((B, S, E), dtype=np.float32)
    for b in range(B):
        out[b] = res.results[2 * b]["out"] + res.results[2 * b + 1]["out"] + bo
    return out


# revision 22
# speedup vs baseline: 1.2835x; 1.2705x over previous
"""Multi-head causal attention (B=4, S=4096, E=512, H=8) on 8 trn2 NeuronCores.

Sharding: core = (batch b, head-group g of 4 heads); 4 batches x 2 groups = 8 cores.
Each core computes qkv projection for its group's heads, causal attention, and a
partial output projection (its heads' rows of Wo). Host sums the two partials per
batch and adds bo.

v2: bf16 operands everywhere (FWL weight loads, 2x SBUF/DMA traffic), QE=512
query sweeps with a merged e/o score tile [128, 1024] so each key-block
iteration is ONE exp activation; double-buffered score PSUM so QK(kb+1)
overlaps exp(kb); projection and Wo work interleaved into the attention
stream to fill PE gaps while the scalar engine (exp) saturates.

Device layout (per core):
  xT   [512, 4096] bf16   x[b] transposed -> contraction dim on partitions
  qT/kT stored [128(2 heads' dh), 1024-token tiles]
  V    stored token-major [128, kb*260 + h*65 + d] bf16 with a ones column per
       (kb, head) at d=64 -> PV matmul lhsT [Vh|1] yields attention output
       in [dh, tok] layout AND softmax denominators in one pass.
  st   [128 keys, 1024] PSUM per key-block: cols 0:512 = even head of the
       pair, 512:1024 = odd head (QK pair runs row-tile concurrent on PE);
       causal mask accumulated on PE via ident@maskT; ONE exp (scale=1/8
       folded) -> pt bf16; PV accumulates over key blocks in PSUM [65, 512].
"""

import sys

sys.path.insert(0, "/opt/trn_rl_repo")

import numpy as np

B, S, E = 4, 4096, 512
H = 8
DH = 64
HPG = 4  # heads per group
GQ = 256  # features per group for each of q/k/v (HPG*DH)
QE = 512  # query extent per attention sweep
NQ = S // QE  # 8
NTQ = 4  # token chunks for projection phase
TQ = S // NTQ  # 1024
VW = HPG * 65  # 260: per-key-block V width incl. ones columns
NEG = -1.0e10
SCALE = 0.125  # 1/sqrt(DH)

_CACHE = {}


def _build_nc():
    import concourse.bass as bass
    import concourse.tile as tile
    import concourse.mybir as mybir
    from concourse import bacc

    f32 = mybir.dt.float32
    f32r = mybir.dt.float32r
    bf = mybir.dt.bfloat16
    AF = mybir.ActivationFunctionType
    ALU = mybir.AluOpType

    nc = bacc.Bacc("TRN2", target_bir_lowering=False, debug=False)

    xT = nc.dram_tensor("xT", [E, S], bf, kind="ExternalInput").ap()
    wqk = nc.dram_tensor("wqk", [E, 512], bf, kind="ExternalInput").ap()
    bqk = nc.dram_tensor("bqk", [128, 4], f32, kind="ExternalInput").ap()
    wv = nc.dram_tensor("wv", [E, GQ], bf, kind="ExternalInput").ap()
    bv = nc.dram_tensor("bv", [1, GQ], bf, kind="ExternalInput").ap()
    wo = nc.dram_tensor("wo", [DH, HPG * 512], bf, kind="ExternalInput").ap()
    out = nc.dram_tensor("out", [S, E], f32, kind="ExternalOutput").ap()

    with tile.TileContext(nc) as tc:
        with (
            tc.tile_pool(name="consts", bufs=1) as cpool,
            tc.tile_pool(name="xt", bufs=2) as xtpool,
            tc.tile_pool(name="qkv", bufs=1) as qkvpool,
            tc.tile_pool(name="pt", bufs=3) as ptpool,
            tc.tile_pool(name="att", bufs=2) as attpool,
            tc.tile_pool(name="eps", bufs=2) as epool,
            tc.tile_pool(name="outs", bufs=2) as opool,
            # PSUM budget (8 banks of 512 f32):
            #   st  [128,1024] x2 bufs = 4 banks
            #   ov_e/ov_o [65,512] x1  = 2 banks
            #   aux [128,512] x2 bufs  = 2 banks (proj + wo matmul groups)
            tc.tile_pool(name="st", bufs=2, space="PSUM") as stpool,
            tc.tile_pool(name="ov", bufs=1, space="PSUM") as ovpool,
            tc.tile_pool(name="aux", bufs=2, space="PSUM") as auxpool,
        ):
            # ---- constants ----
            wqk_sb = cpool.tile([128, 4 * 512], bf, name="wqk_sb")
            for ec in range(4):
                nc.sync.dma_start(
                    wqk_sb[:, ec * 512 : (ec + 1) * 512],
                    wqk[ec * 128 : (ec + 1) * 128, :],
                )
            wv_sb = cpool.tile([128, 4 * GQ], bf, name="wv_sb")
            for ec in range(4):
                nc.sync.dma_start(
                    wv_sb[:, ec * GQ : (ec + 1) * GQ],
                    wv[ec * 128 : (ec + 1) * 128, :],
                )
            wo_sb = cpool.tile([DH, HPG * 512], bf, name="wo_sb")
            nc.sync.dma_start(wo_sb[:], wo[:])
            bqk_sb = cpool.tile([128, 4], f32, name="bqk_sb")
            nc.sync.dma_start(bqk_sb[:], bqk[:])
            bv_sb = cpool.tile([1, GQ], bf, name="bv_sb")
            nc.sync.dma_start(bv_sb[:], bv[:])
            onesf = cpool.tile([128, 128], bf, name="onesf")
            nc.vector.memset(onesf[:], 1.0)
            ones_row = cpool.tile([1, 128], bf, name="ones_row")
            nc.vector.tensor_copy(ones_row[:], onesf[0:1, :])
            ones_f32 = cpool.tile([1, DH], f32, name="ones_f32")
            nc.vector.memset(ones_f32[:], 1.0)
            maskf = cpool.tile([128, 128], f32, name="maskf")
            nc.vector.memset(maskf[:], 0.0)
            nc.gpsimd.affine_select(
                out=maskf[:], in_=maskf[:], compare_op=ALU.is_ge, fill=NEG,
                base=0, pattern=[[1, 128]], channel_multiplier=-1,
            )
            maskT = cpool.tile([128, 128], bf, name="maskT")
            nc.vector.tensor_copy(maskT[:], maskf[:])
            identf = cpool.tile([128, 128], f32, name="identf")
            nc.vector.memset(identf[:], 0.0)
            nc.gpsimd.affine_select(
                out=identf[:], in_=identf[:], compare_op=ALU.not_equal, fill=1.0,
                base=0, pattern=[[-1, 128]], channel_multiplier=1,
            )
            ident = cpool.tile([128, 128], bf, name="ident")
            nc.vector.tensor_copy(ident[:], identf[:])

            # persistent qT/kT tiles: [pair A/B][tq] each [128, 1024] bf16
            # pair A rows 0:64 = head0 dh, 64:128 = head1; pair B = heads 2,3
            qt = [
                [qkvpool.tile([128, TQ], bf, name=f"qt{ab}_{t}") for t in range(NTQ)]
                for ab in range(2)
            ]
            kt = [
                [qkvpool.tile([128, TQ], bf, name=f"kt{ab}_{t}") for t in range(NTQ)]
                for ab in range(2)
            ]
            vt = [qkvpool.tile([128, 8 * VW], bf, name=f"vt_{t}") for t in range(NTQ)]

            # deferred-work queue: small PE work quanta (projection psum groups,
            # Wo psum groups) emitted one per attention kb-iteration so the PE
            # does them while the scalar engine churns exps, instead of in a
            # serial block at sweep boundaries (which idles ACT ~14us each).
            work_q = []  # entries: (label, thunk)

            def drain_one():
                if work_q:
                    work_q.pop(0)[1]()

            def drain_all():
                while work_q:
                    work_q.pop(0)[1]()

            def force_drain(label):
                # emit every queued chunk up to and including the last one
                # with this label (FIFO order preserved for correctness).
                while any(lbl == label for lbl, _ in work_q):
                    work_q.pop(0)[1]()

            def p1(tq, defer):
                xts = []
                for ec in range(4):
                    xtile = xtpool.tile([128, TQ], bf, name="xtile", tag=f"xt{ec}")
                    nc.sync.dma_start(
                        xtile[:],
                        xT[ec * 128 : (ec + 1) * 128, tq * TQ : (tq + 1) * TQ],
                    )
                    xts.append(xtile)

                def qk_chunk(fc, th):
                    dest = (qt if fc < 2 else kt)[fc % 2][tq]
                    ps = auxpool.tile([128, 512], f32, name="pjps", tag="aux")
                    for ec in range(4):
                        nc.tensor.matmul(
                            ps[:],
                            lhsT=wqk_sb[:, ec * 512 + fc * 128 : ec * 512 + (fc + 1) * 128],
                            rhs=xts[ec][:, th * 512 : (th + 1) * 512],
                            start=(ec == 0),
                            stop=(ec == 3),
                        )
                    nc.vector.tensor_scalar_add(
                        dest[:, th * 512 : (th + 1) * 512],
                        ps[:],
                        bqk_sb[:, fc : fc + 1],
                    )

                def v_ones():
                    nc.vector.tensor_copy(
                        vt[tq].rearrange("p (t h d) -> p t h d", t=8, h=HPG)[:, :, :, 64:65],
                        onesf[:, 0:32].rearrange("p (t h d) -> p t h d", t=8, h=HPG),
                    )

                def v_chunk(tb):
                    vps = auxpool.tile([128, GQ], f32, name="vps", tag="aux")
                    for ec in range(4):
                        nc.tensor.matmul(
                            vps[:],
                            lhsT=xts[ec][:, tb * 128 : (tb + 1) * 128],
                            rhs=wv_sb[:, ec * GQ : (ec + 1) * GQ],
                            start=(ec == 0),
                            stop=False,
                        )
                    nc.tensor.matmul(
                        vps[:], lhsT=ones_row[:], rhs=bv_sb[:], start=False, stop=True
                    )
                    nc.vector.tensor_copy(
                        vt[tq][:, tb * VW : (tb + 1) * VW].rearrange(
                            "p (h d) -> p h d", h=HPG
                        )[:, :, 0:64],
                        vps.rearrange("p (h d) -> p h d", h=HPG),
                    )

                chunks = [v_ones]
                chunks += [
                    (lambda fc=fc, th=th: qk_chunk(fc, th))
                    for fc in range(4)
                    for th in range(2)
                ]
                chunks += [(lambda tb=tb: v_chunk(tb)) for tb in range(8)]
                if defer:
                    work_q.extend((f"p1_{tq}", c) for c in chunks)
                else:
                    for c in chunks:
                        c()

            atts = {}

            def pv(kb, pt, qs, pr, nkb, ov_e, ov_o):
                tqk, kbl = kb // 8, kb % 8
                nc.tensor.matmul(
                    ov_e[:, qs:QE],
                    lhsT=vt[tqk][:, kbl * VW + 2 * pr * 65 : kbl * VW + (2 * pr + 1) * 65],
                    rhs=pt[:, qs:QE],
                    start=(kb == 0),
                    stop=(kb == nkb - 1),
                    skip_group_check=True,
                )
                nc.tensor.matmul(
                    ov_o[:, qs:QE],
                    lhsT=vt[tqk][:, kbl * VW + (2 * pr + 1) * 65 : kbl * VW + (2 * pr + 2) * 65],
                    rhs=pt[:, QE + qs : 2 * QE],
                    start=(kb == 0),
                    stop=(kb == nkb - 1),
                    skip_group_check=True,
                )

            def epilogue_inline(pr, ov_e, ov_o):
                # Fast PSUM->SBUF copies so the ov banks free up immediately
                # (the next sweep's first PV has a WAR dependency on them).
                # Everything downstream (recip, broadcast, normalize) works
                # from the SBUF copies and is deferred into the work queue.
                oc_e = epool.tile([65, QE], f32, name="oc_e", tag=f"oc{pr}e")
                oc_o = epool.tile([65, QE], f32, name="oc_o", tag=f"oc{pr}o")
                nc.vector.tensor_copy(oc_e[:], ov_e[:])
                nc.vector.tensor_copy(oc_o[:], ov_o[:])
                return oc_e, oc_o

            def norm_chunks(qq, pr, oc_e, oc_o):
                chunks = []
                for half, oc in ((0, oc_e), (1, oc_o)):
                    h = 2 * pr + half

                    def cn(h=h, oc=oc):
                        den = epool.tile([1, QE], f32, name="den", tag=f"den{h}")
                        nc.vector.tensor_copy(den[:], oc[64:65, :])
                        rec = epool.tile([1, QE], f32, name="rec", tag=f"rec{h}")
                        scr = epool.tile([1, QE], f32, name="scr", tag=f"scr{h}")
                        nc.vector.reciprocal_approx_accurate(
                            out=rec[:], in_=den[:], scratch=scr[:]
                        )
                        # broadcast 1/den across the dh partitions on the PE
                        # (ones outer product) -- a DMA broadcast here has
                        # ~6us latency and would stall the in-order DVE queue.
                        rb = auxpool.tile([DH, QE], f32, name="rb", tag="aux")
                        nc.tensor.matmul(
                            rb[:], lhsT=ones_f32[:], rhs=rec[:], start=True, stop=True
                        )
                        ah = attpool.tile([DH, QE], bf, name=f"att{h}", tag=f"att{h}")
                        nc.vector.tensor_tensor(ah[:], oc[0:DH, :], rb[:], ALU.mult)
                        atts[(qq, h)] = ah

                    chunks.append(cn)
                return chunks

            def wo_chunks(qq):
                out_sb = opool.tile([128, 4 * 512], f32, name="out_sb", tag="osb")

                def wo_chunk(tb4):
                    wops = auxpool.tile([128, 512], f32, name="wops", tag="aux")
                    for h in range(HPG):
                        nc.tensor.matmul(
                            wops[:],
                            lhsT=atts[(qq, h)][:, tb4 * 128 : (tb4 + 1) * 128],
                            rhs=wo_sb[:, h * 512 : (h + 1) * 512],
                            start=(h == 0),
                            stop=(h == HPG - 1),
                        )
                    nc.vector.tensor_copy(out_sb[:, tb4 * 512 : (tb4 + 1) * 512], wops[:])

                def wo_dma():
                    nc.sync.dma_start(
                        out[qq * QE : (qq + 1) * QE, :].rearrange("(t p) c -> p t c", p=128),
                        out_sb.rearrange("p (t c) -> p t c", t=4),
                    )

                return [(lambda tb4=tb4: wo_chunk(tb4)) for tb4 in range(4)] + [wo_dma]

            # One flat stream over all (qq, pr, kb) iterations. The PV matmul
            # is software-pipelined one iteration behind (crossing sweep
            # boundaries), so the in-order PE queue always reaches the next
            # QK -- and the scalar engine's exp stream never starves.
            p1(0, defer=False)
            prev = None
            for qq in range(NQ):
                for pr in range(2):
                    nkb = 4 * qq + 4
                    tqq, qoff = qq // 2, (qq % 2) * QE
                    ov_e = ovpool.tile([65, QE], f32, name="ov_e", tag="ov_e")
                    ov_o = ovpool.tile([65, QE], f32, name="ov_o", tag="ov_o")
                    if pr == 0:
                        # prefetch the NEXT projection chunk set; force-drain
                        # any leftovers of the one this sweep starts reading.
                        tq_next = qq // 2 + 1
                        if qq % 2 == 0 and tq_next < NTQ:
                            p1(tq_next, defer=True)
                        if qq % 2 == 0 and qq > 0:
                            force_drain(f"p1_{qq // 2}")
                    for kb in range(nkb):
                        tqk, kbl = kb // 8, kb % 8
                        qs = max(0, kb * 128 - qq * QE)
                        diag = kb >= 4 * qq
                        st = stpool.tile([128, 2 * QE], f32, name="st", tag="st")
                        # QK^T for the head pair: rows 0:64 (even) / 64:128
                        # (odd) use disjoint PE row groups -> concurrent.
                        nc.tensor.matmul(
                            st[:, qs:QE],
                            lhsT=kt[pr][tqk][0:64, kbl * 128 : (kbl + 1) * 128],
                            rhs=qt[pr][tqq][0:64, qoff + qs : qoff + QE],
                            start=True,
                            stop=not diag,
                        )
                        nc.tensor.matmul(
                            st[:, QE + qs : 2 * QE],
                            lhsT=kt[pr][tqk][64:128, kbl * 128 : (kbl + 1) * 128],
                            rhs=qt[pr][tqq][64:128, qoff + qs : qoff + QE],
                            start=True,
                            stop=not diag,
                        )
                        if diag:
                            for half in range(2):
                                nc.tensor.matmul(
                                    st[:, half * QE + qs : half * QE + qs + 128],
                                    lhsT=ident[:],
                                    rhs=maskT[:],
                                    start=False,
                                    stop=True,
                                    skip_group_check=True,
                                )
                        pt = ptpool.tile([128, 2 * QE], bf, name="pt", tag="pt")
                        if qs == 0:
                            nc.scalar.activation(
                                pt[:], st[:], AF.Exp, bias=0.0, scale=SCALE
                            )
                        else:
                            st3 = st.rearrange("p (t c) -> p t c", t=2)[:, :, qs:QE]
                            pt3 = pt.rearrange("p (t c) -> p t c", t=2)[:, :, qs:QE]
                            nc.scalar.activation(
                                pt3, st3, AF.Exp, bias=0.0, scale=SCALE
                            )
                        if prev is not None:
                            pv(*prev[0])
                            if prev[1] is not None:
                                # prev iteration closed a sweep: free its ov
                                # banks now, defer normalize + Wo projection.
                                eqq, epr, eov_e, eov_o = prev[1]
                                oc_e, oc_o = epilogue_inline(epr, eov_e, eov_o)
                                work_q.extend(
                                    ("ep", c)
                                    for c in norm_chunks(eqq, epr, oc_e, oc_o)
                                )
                                if epr == 1:
                                    work_q.extend(
                                        ("wo", c) for c in wo_chunks(eqq)
                                    )
                        done = (
                            (qq, pr, ov_e, ov_o) if kb == nkb - 1 else None
                        )
                        prev = ((kb, pt, qs, pr, nkb, ov_e, ov_o), done)
                        drain_one()
            pv(*prev[0])
            eqq, epr, eov_e, eov_o = prev[1]
            oc_e, oc_o = epilogue_inline(epr, eov_e, eov_o)
            for c in norm_chunks(eqq, epr, oc_e, oc_o):
                c()
            for c in wo_chunks(eqq):
                c()
            drain_all()

    nc.finalize()
    return nc


def _get_nc():
    if "nc" not in _CACHE:
        _CACHE["nc"] = _build_nc()
    return _CACHE["nc"]


def _make_in_maps(x, Wqkv, bqkv, Wo):
    import ml_dtypes

    bf16 = ml_dtypes.bfloat16
    in_maps = []
    for core in range(8):
        b, g = core // 2, core % 2
        qs, ks, vs = g * GQ, 512 + g * GQ, 1024 + g * GQ
        wqk_np = np.ascontiguousarray(
            np.concatenate([Wqkv[:, qs : qs + GQ], Wqkv[:, ks : ks + GQ]], axis=1)
        ).astype(bf16)
        bqk_np = np.ascontiguousarray(
            np.concatenate([bqkv[qs : qs + GQ], bqkv[ks : ks + GQ]]).reshape(4, 128).T
        )
        wv_np = np.ascontiguousarray(Wqkv[:, vs : vs + GQ]).astype(bf16)
        bv_np = np.ascontiguousarray(bqkv[vs : vs + GQ].reshape(1, GQ)).astype(bf16)
        wo_g = Wo[g * GQ : (g + 1) * GQ, :]
        wo_np = np.ascontiguousarray(
            np.concatenate([wo_g[h * DH : (h + 1) * DH, :] for h in range(HPG)], axis=1)
        ).astype(bf16)
        in_maps.append(
            {
                "xT": np.ascontiguousarray(x[b].T).astype(bf16),
                "wqk": wqk_np,
                "bqk": bqk_np,
                "wv": wv_np,
                "bv": bv_np,
                "wo": wo_np,
            }
        )
    return in_maps


def kernel(x, Wqkv, bqkv, Wo, bo, **run_kwargs):
    from concourse.bass_utils import run_bass_kernel_spmd

    x = np.asarray(x, dtype=np.float32)
    Wqkv = np.asarray(Wqkv, dtype=np.float32)
    bqkv = np.asarray(bqkv, dtype=np.float32)
    Wo = np.asarray(Wo, dtype=np.float32)
    bo = np.asarray(bo, dtype=np.float32)

    nc = _get_nc()
    in_maps = _make_in_maps(x, Wqkv, bqkv, Wo)

    res = run_bass_kernel_spmd(nc, in_maps, core_ids=list(range(8)), **run_kwargs)
    _CACHE["last_results"] = res

    out = np.empty((B, S, E), dtype=np.float32)
    for b in range(B):
        out[b] = res.results[2 * b]["out"] + res.results[2 * b + 1]["out"] + bo
    return out


# revision 28
# speedup vs baseline: 1.2864x; 1.0022x over previous
"""Multi-head causal attention (B=4, S=4096, E=512, H=8) on 8 trn2 NeuronCores.

Sharding: core = (batch b, head-group g of 4 heads); 4 batches x 2 groups = 8 cores.
Each core computes qkv projection for its group's heads, causal attention, and a
partial output projection (its heads' rows of Wo). Host sums the two partials per
batch and adds bo.

v2: bf16 operands everywhere (FWL weight loads, 2x SBUF/DMA traffic), QE=512
query sweeps with a merged e/o score tile [128, 1024] so each key-block
iteration is ONE exp activation; double-buffered score PSUM so QK(kb+1)
overlaps exp(kb); projection and Wo work interleaved into the attention
stream to fill PE gaps while the scalar engine (exp) saturates.

Device layout (per core):
  xT   [512, 4096] bf16   x[b] transposed -> contraction dim on partitions
  qT/kT stored [128(2 heads' dh), 1024-token tiles]
  V    stored token-major [128, kb*260 + h*65 + d] bf16 with a ones column per
       (kb, head) at d=64 -> PV matmul lhsT [Vh|1] yields attention output
       in [dh, tok] layout AND softmax denominators in one pass.
  st   [128 keys, 1024] PSUM per key-block: cols 0:512 = even head of the
       pair, 512:1024 = odd head (QK pair runs row-tile concurrent on PE);
       causal mask accumulated on PE via ident@maskT; ONE exp (scale=1/8
       folded) -> pt bf16; PV accumulates over key blocks in PSUM [65, 512].
"""

import sys

sys.path.insert(0, "/opt/trn_rl_repo")

import numpy as np

B, S, E = 4, 4096, 512
H = 8
DH = 64
HPG = 4  # heads per group
GQ = 256  # features per group for each of q/k/v (HPG*DH)
QE = 512  # query extent per attention sweep
NQ = S // QE  # 8
NTQ = 4  # token chunks for projection phase
TQ = S // NTQ  # 1024
VW = HPG * 65  # 260: per-key-block V width incl. ones columns
NEG = -1.0e10
SCALE = 0.125  # 1/sqrt(DH)

_CACHE = {}


def _build_nc():
    import concourse.bass as bass
    import concourse.tile as tile
    import concourse.mybir as mybir
    from concourse import bacc

    f32 = mybir.dt.float32
    f32r = mybir.dt.float32r
    bf = mybir.dt.bfloat16
    AF = mybir.ActivationFunctionType
    ALU = mybir.AluOpType

    nc = bacc.Bacc("TRN2", target_bir_lowering=False, debug=False)

    xT = nc.dram_tensor("xT", [E, S], bf, kind="ExternalInput").ap()
    wqk = nc.dram_tensor("wqk", [E, 512], bf, kind="ExternalInput").ap()
    bqk = nc.dram_tensor("bqk", [128, 4], f32, kind="ExternalInput").ap()
    wv = nc.dram_tensor("wv", [E, GQ], bf, kind="ExternalInput").ap()
    bv = nc.dram_tensor("bv", [1, GQ], bf, kind="ExternalInput").ap()
    wo = nc.dram_tensor("wo", [DH, HPG * 512], bf, kind="ExternalInput").ap()
    out = nc.dram_tensor("out", [S, E], f32, kind="ExternalOutput").ap()

    with tile.TileContext(nc) as tc:
        with (
            tc.tile_pool(name="xt", bufs=2) as xtpool,
            tc.tile_pool(name="consts", bufs=1) as cpool,
            tc.tile_pool(name="qkv", bufs=1) as qkvpool,
            tc.tile_pool(name="pt", bufs=3) as ptpool,
            tc.tile_pool(name="att", bufs=2) as attpool,
            tc.tile_pool(name="eps", bufs=2) as epool,
            tc.tile_pool(name="outs", bufs=2) as opool,
            # PSUM budget (8 banks of 512 f32):
            #   st  [128,1024] x2 bufs = 4 banks
            #   ov_e/ov_o [65,512] x1  = 2 banks
            #   aux [128,512] x2 bufs  = 2 banks (proj + wo matmul groups)
            tc.tile_pool(name="st", bufs=2, space="PSUM") as stpool,
            tc.tile_pool(name="ov", bufs=1, space="PSUM") as ovpool,
            tc.tile_pool(name="aux", bufs=2, space="PSUM") as auxpool,
        ):
            # prefetch the first projection chunk's x before the weights so
            # the PE's first matmul isn't stuck behind the weight DMAs.
            xts0 = []
            for ec in range(4):
                xtile = xtpool.tile([128, TQ], bf, name="xtile", tag=f"xt{ec}")
                nc.sync.dma_start(xtile[:], xT[ec * 128 : (ec + 1) * 128, 0:TQ])
                xts0.append(xtile)

            # ---- constants ----
            wqk_sb = cpool.tile([128, 4 * 512], bf, name="wqk_sb")
            for ec in range(4):
                nc.sync.dma_start(
                    wqk_sb[:, ec * 512 : (ec + 1) * 512],
                    wqk[ec * 128 : (ec + 1) * 128, :],
                )
            wv_sb = cpool.tile([128, 4 * GQ], bf, name="wv_sb")
            for ec in range(4):
                nc.sync.dma_start(
                    wv_sb[:, ec * GQ : (ec + 1) * GQ],
                    wv[ec * 128 : (ec + 1) * 128, :],
                )
            wo_sb = cpool.tile([DH, HPG * 512], bf, name="wo_sb")
            nc.sync.dma_start(wo_sb[:], wo[:])
            bqk_sb = cpool.tile([128, 4], f32, name="bqk_sb")
            nc.sync.dma_start(bqk_sb[:], bqk[:])
            bv_sb = cpool.tile([1, GQ], bf, name="bv_sb")
            nc.sync.dma_start(bv_sb[:], bv[:])
            onesf = cpool.tile([128, 128], bf, name="onesf")
            nc.vector.memset(onesf[:], 1.0)
            ones_row = cpool.tile([1, 128], bf, name="ones_row")
            nc.vector.tensor_copy(ones_row[:], onesf[0:1, :])
            ones_f32 = cpool.tile([1, DH], f32, name="ones_f32")
            nc.vector.memset(ones_f32[:], 1.0)
            # persistent qT/kT tiles: [pair A/B][tq] each [128, 1024] bf16
            # pair A rows 0:64 = head0 dh, 64:128 = head1; pair B = heads 2,3
            qt = [
                [qkvpool.tile([128, TQ], bf, name=f"qt{ab}_{t}") for t in range(NTQ)]
                for ab in range(2)
            ]
            kt = [
                [qkvpool.tile([128, TQ], bf, name=f"kt{ab}_{t}") for t in range(NTQ)]
                for ab in range(2)
            ]
            vt = [qkvpool.tile([128, 8 * VW], bf, name=f"vt_{t}") for t in range(NTQ)]

            # deferred-work queue: small PE work quanta (projection psum groups,
            # Wo psum groups) emitted one per attention kb-iteration so the PE
            # does them while the scalar engine churns exps, instead of in a
            # serial block at sweep boundaries (which idles ACT ~14us each).
            work_q = []  # entries: (label, thunk)

            def drain_one():
                if work_q:
                    work_q.pop(0)[1]()

            def drain_all():
                while work_q:
                    work_q.pop(0)[1]()

            def force_drain(label):
                # emit every queued chunk up to and including the last one
                # with this label (FIFO order preserved for correctness).
                while any(lbl == label for lbl, _ in work_q):
                    work_q.pop(0)[1]()

            def p1(tq, defer, pre_xts=None):
                if pre_xts is not None:
                    xts = pre_xts
                else:
                    xts = []
                    for ec in range(4):
                        xtile = xtpool.tile([128, TQ], bf, name="xtile", tag=f"xt{ec}")
                        nc.sync.dma_start(
                            xtile[:],
                            xT[ec * 128 : (ec + 1) * 128, tq * TQ : (tq + 1) * TQ],
                        )
                        xts.append(xtile)

                def qk_chunk(fc, th):
                    dest = (qt if fc < 2 else kt)[fc % 2][tq]
                    ps = auxpool.tile([128, 512], f32, name="pjps", tag="aux")
                    for ec in range(4):
                        nc.tensor.matmul(
                            ps[:],
                            lhsT=wqk_sb[:, ec * 512 + fc * 128 : ec * 512 + (fc + 1) * 128],
                            rhs=xts[ec][:, th * 512 : (th + 1) * 512],
                            start=(ec == 0),
                            stop=(ec == 3),
                        )
                    nc.vector.tensor_scalar_add(
                        dest[:, th * 512 : (th + 1) * 512],
                        ps[:],
                        bqk_sb[:, fc : fc + 1],
                    )

                def v_ones():
                    nc.vector.tensor_copy(
                        vt[tq].rearrange("p (t h d) -> p t h d", t=8, h=HPG)[:, :, :, 64:65],
                        onesf[:, 0:32].rearrange("p (t h d) -> p t h d", t=8, h=HPG),
                    )

                def v_chunk(tb):
                    vps = auxpool.tile([128, GQ], f32, name="vps", tag="aux")
                    for ec in range(4):
                        nc.tensor.matmul(
                            vps[:],
                            lhsT=xts[ec][:, tb * 128 : (tb + 1) * 128],
                            rhs=wv_sb[:, ec * GQ : (ec + 1) * GQ],
                            start=(ec == 0),
                            stop=False,
                        )
                    nc.tensor.matmul(
                        vps[:], lhsT=ones_row[:], rhs=bv_sb[:], start=False, stop=True
                    )
                    nc.vector.tensor_copy(
                        vt[tq][:, tb * VW : (tb + 1) * VW].rearrange(
                            "p (h d) -> p h d", h=HPG
                        )[:, :, 0:64],
                        vps.rearrange("p (h d) -> p h d", h=HPG),
                    )

                chunks = [v_ones]
                chunks += [
                    (lambda fc=fc, th=th: qk_chunk(fc, th))
                    for fc in range(4)
                    for th in range(2)
                ]
                chunks += [(lambda tb=tb: v_chunk(tb)) for tb in range(8)]
                if defer:
                    work_q.extend((f"p1_{tq}", c) for c in chunks)
                else:
                    for c in chunks:
                        c()

            atts = {}

            def pv(kb, pt, qs, pr, nkb, ov_e, ov_o):
                tqk, kbl = kb // 8, kb % 8
                nc.tensor.matmul(
                    ov_e[:, qs:QE],
                    lhsT=vt[tqk][:, kbl * VW + 2 * pr * 65 : kbl * VW + (2 * pr + 1) * 65],
                    rhs=pt[:, qs:QE],
                    start=(kb == 0),
                    stop=(kb == nkb - 1),
                    skip_group_check=True,
                )
                nc.tensor.matmul(
                    ov_o[:, qs:QE],
                    lhsT=vt[tqk][:, kbl * VW + (2 * pr + 1) * 65 : kbl * VW + (2 * pr + 2) * 65],
                    rhs=pt[:, QE + qs : 2 * QE],
                    start=(kb == 0),
                    stop=(kb == nkb - 1),
                    skip_group_check=True,
                )

            def epilogue_inline(pr, ov_e, ov_o):
                # Fast PSUM->SBUF copies so the ov banks free up immediately
                # (the next sweep's first PV has a WAR dependency on them).
                # Everything downstream (recip, broadcast, normalize) works
                # from the SBUF copies and is deferred into the work queue.
                oc_e = epool.tile([65, QE], f32, name="oc_e", tag=f"oc{pr}e")
                oc_o = epool.tile([65, QE], f32, name="oc_o", tag=f"oc{pr}o")
                nc.vector.tensor_copy(oc_e[:], ov_e[:])
                nc.vector.tensor_copy(oc_o[:], ov_o[:])
                return oc_e, oc_o

            def norm_chunks(qq, pr, oc_e, oc_o):
                chunks = []
                for half, oc in ((0, oc_e), (1, oc_o)):
                    h = 2 * pr + half

                    def cn(h=h, oc=oc):
                        den = epool.tile([1, QE], f32, name="den", tag=f"den{h}")
                        nc.vector.tensor_copy(den[:], oc[64:65, :])
                        rec = epool.tile([1, QE], f32, name="rec", tag=f"rec{h}")
                        scr = epool.tile([1, QE], f32, name="scr", tag=f"scr{h}")
                        nc.vector.reciprocal_approx_accurate(
                            out=rec[:], in_=den[:], scratch=scr[:]
                        )
                        # broadcast 1/den across the dh partitions on the PE
                        # (ones outer product) -- a DMA broadcast here has
                        # ~6us latency and would stall the in-order DVE queue.
                        rb = auxpool.tile([DH, QE], f32, name="rb", tag="aux")
                        nc.tensor.matmul(
                            rb[:], lhsT=ones_f32[:], rhs=rec[:], start=True, stop=True
                        )
                        ah = attpool.tile([DH, QE], bf, name=f"att{h}", tag=f"att{h}")
                        nc.vector.tensor_tensor(ah[:], oc[0:DH, :], rb[:], ALU.mult)
                        atts[(qq, h)] = ah

                    chunks.append(cn)
                return chunks

            def wo_chunks(qq):
                out_sb = opool.tile([128, 4 * 512], f32, name="out_sb", tag="osb")

                def wo_chunk(tb4):
                    wops = auxpool.tile([128, 512], f32, name="wops", tag="aux")
                    for h in range(HPG):
                        nc.tensor.matmul(
                            wops[:],
                            lhsT=atts[(qq, h)][:, tb4 * 128 : (tb4 + 1) * 128],
                            rhs=wo_sb[:, h * 512 : (h + 1) * 512],
                            start=(h == 0),
                            stop=(h == HPG - 1),
                        )
                    nc.vector.tensor_copy(out_sb[:, tb4 * 512 : (tb4 + 1) * 512], wops[:])

                def wo_dma():
                    nc.sync.dma_start(
                        out[qq * QE : (qq + 1) * QE, :].rearrange("(t p) c -> p t c", p=128),
                        out_sb.rearrange("p (t c) -> p t c", t=4),
                    )

                return [(lambda tb4=tb4: wo_chunk(tb4)) for tb4 in range(4)] + [wo_dma]

            # One flat stream over all (qq, pr, kb) iterations. The PV matmul
            # is software-pipelined one iteration behind (crossing sweep
            # boundaries), so the in-order PE queue always reaches the next
            # QK -- and the scalar engine's exp stream never starves.
            p1(0, defer=False, pre_xts=xts0)
            prev = None
            for qq in range(NQ):
                for pr in range(2):
                    nkb = 4 * qq + 4
                    tqq, qoff = qq // 2, (qq % 2) * QE
                    ov_e = ovpool.tile([65, QE], f32, name="ov_e", tag="ov_e")
                    ov_o = ovpool.tile([65, QE], f32, name="ov_o", tag="ov_o")
                    if pr == 0:
                        # prefetch the NEXT projection chunk set; force-drain
                        # any leftovers of the one this sweep starts reading.
                        tq_next = qq // 2 + 1
                        if qq % 2 == 0 and tq_next < NTQ:
                            p1(tq_next, defer=True)
                        if qq % 2 == 0 and qq > 0:
                            force_drain(f"p1_{qq // 2}")
                    for kb in range(nkb):
                        tqk, kbl = kb // 8, kb % 8
                        qs = max(0, kb * 128 - qq * QE)
                        diag = kb >= 4 * qq
                        st = stpool.tile([128, 2 * QE], f32, name="st", tag="st")
                        # QK^T for the head pair: rows 0:64 (even) / 64:128
                        # (odd) use disjoint PE row groups -> concurrent.
                        nc.tensor.matmul(
                            st[:, qs:QE],
                            lhsT=kt[pr][tqk][0:64, kbl * 128 : (kbl + 1) * 128],
                            rhs=qt[pr][tqq][0:64, qoff + qs : qoff + QE],
                            start=True,
                            stop=True,
                        )
                        nc.tensor.matmul(
                            st[:, QE + qs : 2 * QE],
                            lhsT=kt[pr][tqk][64:128, kbl * 128 : (kbl + 1) * 128],
                            rhs=qt[pr][tqq][64:128, qoff + qs : qoff + QE],
                            start=True,
                            stop=True,
                        )
                        pt = ptpool.tile([128, 2 * QE], bf, name="pt", tag="pt")
                        if qs == 0:
                            nc.scalar.activation(
                                pt[:], st[:], AF.Exp, bias=0.0, scale=SCALE
                            )
                        else:
                            st3 = st.rearrange("p (t c) -> p t c", t=2)[:, :, qs:QE]
                            pt3 = pt.rearrange("p (t c) -> p t c", t=2)[:, :, qs:QE]
                            nc.scalar.activation(
                                pt3, st3, AF.Exp, bias=0.0, scale=SCALE
                            )
                        if diag:
                            # causal mask: zero the below-diagonal triangle of
                            # the diagonal 128x128 block on the (idle) GPSIMD
                            # engine instead of a -1e10 mask matmul on the PE.
                            for half in range(2):
                                blk = pt[:, half * QE + qs : half * QE + qs + 128]
                                nc.gpsimd.affine_select(
                                    out=blk,
                                    in_=blk,
                                    compare_op=ALU.is_ge,
                                    fill=0.0,
                                    base=0,
                                    pattern=[[1, 128]],
                                    channel_multiplier=-1,
                                )
                        if prev is not None:
                            pv(*prev[0])
                            if prev[1] is not None:
                                # prev iteration closed a sweep: free its ov
                                # banks now, defer normalize + Wo projection.
                                eqq, epr, eov_e, eov_o = prev[1]
                                oc_e, oc_o = epilogue_inline(epr, eov_e, eov_o)
                                work_q.extend(
                                    ("ep", c)
                                    for c in norm_chunks(eqq, epr, oc_e, oc_o)
                                )
                                if epr == 1:
                                    work_q.extend(
                                        ("wo", c) for c in wo_chunks(eqq)
                                    )
                        done = (
                            (qq, pr, ov_e, ov_o) if kb == nkb - 1 else None
                        )
                        prev = ((kb, pt, qs, pr, nkb, ov_e, ov_o), done)
                        drain_one()
            pv(*prev[0])
            eqq, epr, eov_e, eov_o = prev[1]
            oc_e, oc_o = epilogue_inline(epr, eov_e, eov_o)
            for c in norm_chunks(eqq, epr, oc_e, oc_o):
                c()
            for c in wo_chunks(eqq):
                c()
            drain_all()

    nc.finalize()
    return nc


def _get_nc():
    if "nc" not in _CACHE:
        _CACHE["nc"] = _build_nc()
    return _CACHE["nc"]


def _make_in_maps(x, Wqkv, bqkv, Wo):
    import ml_dtypes

    bf16 = ml_dtypes.bfloat16
    in_maps = []
    for core in range(8):
        b, g = core // 2, core % 2
        qs, ks, vs = g * GQ, 512 + g * GQ, 1024 + g * GQ
        wqk_np = np.ascontiguousarray(
            np.concatenate([Wqkv[:, qs : qs + GQ], Wqkv[:, ks : ks + GQ]], axis=1)
        ).astype(bf16)
        bqk_np = np.ascontiguousarray(
            np.concatenate([bqkv[qs : qs + GQ], bqkv[ks : ks + GQ]]).reshape(4, 128).T
        )
        wv_np = np.ascontiguousarray(Wqkv[:, vs : vs + GQ]).astype(bf16)
        bv_np = np.ascontiguousarray(bqkv[vs : vs + GQ].reshape(1, GQ)).astype(bf16)
        wo_g = Wo[g * GQ : (g + 1) * GQ, :]
        wo_np = np.ascontiguousarray(
            np.concatenate([wo_g[h * DH : (h + 1) * DH, :] for h in range(HPG)], axis=1)
        ).astype(bf16)
        in_maps.append(
            {
                "xT": np.ascontiguousarray(x[b].T).astype(bf16),
                "wqk": wqk_np,
                "bqk": bqk_np,
                "wv": wv_np,
                "bv": bv_np,
                "wo": wo_np,
            }
        )
    return in_maps


def kernel(x, Wqkv, bqkv, Wo, bo, **run_kwargs):
    from concourse.bass_utils import run_bass_kernel_spmd

    x = np.asarray(x, dtype=np.float32)
    Wqkv = np.asarray(Wqkv, dtype=np.float32)
    bqkv = np.asarray(bqkv, dtype=np.float32)
    Wo = np.asarray(Wo, dtype=np.float32)
    bo = np.asarray(bo, dtype=np.float32)

    nc = _get_nc()
    in_maps = _make_in_maps(x, Wqkv, bqkv, Wo)

    res = run_bass_kernel_spmd(nc, in_maps, core_ids=list(range(8)), **run_kwargs)
    _CACHE["last_results"] = res

    out = np.empty((B, S, E), dtype=np.float32)
    for b in range(B):
        out[b] = res.results[2 * b]["out"] + res.results[2 * b + 1]["out"] + bo
    return out
